# revision 30
# baseline (speedup 1.0000x reference)
"""Trainium2 Bass kernel for nn_ExpertParallelWrapper (MoE top-2 routing, 8 experts,
shared expert), expert-parallel across 8 NeuronCores.

v2 rewrite of the baseline. Key changes vs baseline:
  - Dispatch-list construction no longer uses 128 serialized element-granular
    indirect DMAs (~1.7 ms). Instead: per-chunk stream compaction via one-hot
    matmuls on the PE (list position -> (token_id, weight, hit)), per-chunk
    counts padded to multiples of 4, then 12 chained indirect DMAs per list
    tensor whose 4-slot descriptors are byte-disjoint (order-free); inactive
    sub-runs are skipped via bounds_check + oob_is_err=False. Empty slots point
    at the trash row T with weight 0.
  - Emission order overlaps everything: gating -> AllGather -> dispatch build
    (+scatters) -> shared-expert blocks 0,1 (hide AG + scatter-chain latency)
    -> expert FFN -> ReduceScatter -> shared blocks 2,3 (hide RS) -> combine.
  - Gate and shared-gate matmuls merged into one [H,9] rhs.
  - Shared-expert weights are streamed in slices (no 12.6 MB residency), the
    half outputs accumulate in held PSUM banks, y goes to DRAM until combine.
  - Capacity C=4608 (pad-4 per-chunk counts measured 4412 max + margin).

kernel(**inputs) takes the full unsharded inputs and returns the full output.
"""

import os
import numpy as np

# ---------------- problem sizes (hardcoded per contract) ----------------
B, S, H = 4, 4096, 1024
E, I, IS = 8, 2048, 4096
NCORES = 8
T = B * S                     # 16384 tokens
TLOC = T // NCORES            # 2048 tokens per core
P = 128
C = 4608                      # expert capacity (36*128), pad-4 counts max 4412
NT2 = C // P                  # 36 slot-chunks of 128
KH = H // P                   # 8  k-tiles over H
KI = I // P                   # 16 k-tiles over I
NCH = TLOC // P               # 16 gating chunks per core
BIGI = 1 << 26                # skip-marker offset (> C, exact in f32)
G_SC = 12                     # scatter sub-run instructions (4 slots each)
FFN_BLOCKS = [(i * 512, 512) for i in range(9)]
SH_TB = 512                   # shared-expert token block
SH_NB = TLOC // SH_TB         # 4 shared blocks

_RUNNER = {}
LAST_RESULT = None
LAST_WALL_NS = None


def _f32(a):
    return np.ascontiguousarray(np.asarray(a, dtype=np.float32))


def _bf16(a):
    import ml_dtypes
    return np.ascontiguousarray(np.asarray(a).astype(ml_dtypes.bfloat16))


def build_program(collectives=True):
    import concourse.bass as bass
    import concourse.bacc as bacc
    import concourse.mybir as mybir
    import concourse.tile as tile
    from contextlib import ExitStack

    f32 = mybir.dt.float32
    bf16 = mybir.dt.bfloat16
    i32 = mybir.dt.int32
    AF = mybir.ActivationFunctionType
    ALU = mybir.AluOpType
    X = mybir.AxisListType.X

    nc = bacc.Bacc(None, num_devices=NCORES)
    groups = [list(range(NCORES))]

    # ---------------- I/O ----------------
    xt_f32 = nc.dram_tensor("xt_f32", [H, TLOC], f32, kind="ExternalInput")
    xt_bf = nc.dram_tensor("xt_bf", [H, TLOC], bf16, kind="ExternalInput")
    x_rows = nc.dram_tensor("x_rows", [T + 1, H], bf16, kind="ExternalInput")
    gw9 = nc.dram_tensor("gw9", [H, 9], f32, kind="ExternalInput")
    w1 = nc.dram_tensor("w1", [H, I], bf16, kind="ExternalInput")
    w3 = nc.dram_tensor("w3", [H, I], bf16, kind="ExternalInput")
    w2 = nc.dram_tensor("w2", [I, H], bf16, kind="ExternalInput")
    sw1 = nc.dram_tensor("sw1", [H, IS], bf16, kind="ExternalInput")
    sw3 = nc.dram_tensor("sw3", [H, IS], bf16, kind="ExternalInput")
    sw2 = nc.dram_tensor("sw2", [IS, H], bf16, kind="ExternalInput")
    eid = nc.dram_tensor("eid", [P, 1], f32, kind="ExternalInput")
    out = nc.dram_tensor("out", [TLOC, H], f32, kind="ExternalOutput")

    # internal DRAM
    meta_local = nc.dram_tensor("meta_local", [NCH, 4 * P], f32)
    meta_all = nc.dram_tensor("meta_all", [NCORES * NCH, 4 * P], f32,
                              addr_space="Shared")
    list_id = nc.dram_tensor("list_id", [C, 1], f32)
    list_w = nc.dram_tensor("list_w", [C, 1], f32)
    partial = nc.dram_tensor("partial", [T + 1, H], bf16)
    rs_out = nc.dram_tensor("rs_out", [TLOC, H], bf16)
    yacc_d = nc.dram_tensor("yacc_d", [TLOC, H], bf16)

    # ---------------- inline constants ----------------
    import ml_dtypes
    ident_bf_c = nc.inline_tensor(
        np.eye(P, dtype=np.float32).astype(ml_dtypes.bfloat16), name="ident_bf")
    ident_f_c = nc.inline_tensor(np.eye(P, dtype=np.float32), name="ident_f")
    lts_c = nc.inline_tensor(np.triu(np.ones((P, P), np.float32), 1), name="lts")
    r64_c = nc.inline_tensor(
        np.tile(np.arange(64, dtype=np.float32), (P, 1)), name="r64")
    r128_c = nc.inline_tensor(
        np.tile(np.arange(P, dtype=np.float32), (P, 1)), name="r128")
    tab8_c = nc.inline_tensor(
        np.tile((np.ceil(np.arange(P) / 4.0) * 4.0).astype(np.float32), (P, 1)),
        name="tab8")
    g8_c = nc.inline_tensor(
        np.tile((4.0 * np.arange(G_SC, dtype=np.float32)), (P, 1)), name="g8")
    # rhs3 plane 0 = token ids: ids[t, 3c] = c*128 + t
    rhs3_np = np.zeros((P, P, 3), np.float32)
    rhs3_np[:, :, 0] = (np.arange(P, dtype=np.float32)[None, :] * P
                        + np.arange(P, dtype=np.float32)[:, None])
    rhs3_c = nc.inline_tensor(rhs3_np.reshape(P, 3 * P), name="rhs3c")

    with tile.TileContext(nc) as tc, ExitStack() as ctx:
        const = ctx.enter_context(tc.tile_pool(name="const", bufs=1))

        id_bf = const.tile([P, P], bf16)
        nc.sync.dma_start(out=id_bf[:], in_=ident_bf_c[:, :])
        id_f = const.tile([P, P], f32)
        nc.sync.dma_start(out=id_f[:], in_=ident_f_c[:, :])
        lts = const.tile([P, P], f32)
        nc.sync.dma_start(out=lts[:], in_=lts_c[:, :])
        r64 = const.tile([P, 64], f32)
        nc.sync.dma_start(out=r64[:], in_=r64_c[:, :])
        r128 = const.tile([P, P], f32)
        nc.sync.dma_start(out=r128[:], in_=r128_c[:, :])
        tab8 = const.tile([P, P], f32)
        nc.sync.dma_start(out=tab8[:], in_=tab8_c[:, :])
        g8r = const.tile([P, G_SC], f32)
        nc.sync.dma_start(out=g8r[:], in_=g8_c[:, :])
        eid_sb = const.tile([P, 1], f32)
        nc.sync.dma_start(out=eid_sb[:], in_=eid[:, :])
        sg_all = const.tile([P, NCH], f32)
        lm_id = const.tile([P, NT2], i32)
        lm_w = const.tile([P, NT2], f32)

        # ---- list sentinel init (tiny; must precede the dispatch scatters) ----
        init_ctx = ExitStack()
        initp = init_ctx.enter_context(tc.tile_pool(name="initp", bufs=1))
        sent_id = initp.tile([P, NT2], f32)
        nc.vector.memset(sent_id[:], float(T))
        nc.sync.dma_start(out=list_id[:, :], in_=sent_id[:])
        sent_w = initp.tile([P, NT2], f32)
        nc.vector.memset(sent_w[:], 0.0)
        nc.sync.dma_start(out=list_w[:, :], in_=sent_w[:])
        init_ctx.close()

        # ---- expert weights resident (loads overlap gating) ----
        wexp13_ctx = ExitStack()
        wexp13 = wexp13_ctx.enter_context(tc.tile_pool(name="wexp13", bufs=1))
        wexp2_ctx = ExitStack()
        wexp2 = wexp2_ctx.enter_context(tc.tile_pool(name="wexp2", bufs=1))
        w1_sb = wexp13.tile([P, KH, I], bf16)
        w3_sb = wexp13.tile([P, KH, I], bf16)
        w2_sb = wexp2.tile([P, KI, H], bf16)
        for k in range(KH):
            nc.sync.dma_start(out=w1_sb[:, k, :], in_=w1[k * P:(k + 1) * P, :])
            nc.sync.dma_start(out=w3_sb[:, k, :], in_=w3[k * P:(k + 1) * P, :])
        for k in range(KI):
            nc.sync.dma_start(out=w2_sb[:, k, :], in_=w2[k * P:(k + 1) * P, :])

        # =================== Phase 1: gating (fp32, PE) ===================
        gate_ctx = ExitStack()
        gpool = gate_ctx.enter_context(tc.tile_pool(name="gate", bufs=1))
        gx = gate_ctx.enter_context(tc.tile_pool(name="gx", bufs=2))
        gwork = gate_ctx.enter_context(tc.tile_pool(name="gwork", bufs=4))
        psum_g = gate_ctx.enter_context(tc.tile_pool(name="psum_g", bufs=1,
                                                     space="PSUM"))
        gw9_sb = gpool.tile([P, KH, 9], f32)
        for k in range(KH):
            nc.sync.dma_start(out=gw9_sb[:, k, :], in_=gw9[k * P:(k + 1) * P, :])

        ps9all = psum_g.tile([P, NCH, 9], f32)
        for h in range(2):
            hsl = slice(h * (TLOC // 2), (h + 1) * (TLOC // 2))
            xtf = gx.tile([P, KH, TLOC // 2], f32, tag="xtf")
            for k in range(KH):
                nc.sync.dma_start(out=xtf[:, k, :], in_=xt_f32[k * P:(k + 1) * P, hsl])
            for j in range(NCH // 2):
                jj = h * (NCH // 2) + j
                sl = slice(j * P, (j + 1) * P)
                for k in range(KH):
                    nc.tensor.matmul(out=ps9all[:, jj, :], lhsT=xtf[:, k, sl],
                                     rhs=gw9_sb[:, k, :],
                                     start=(k == 0), stop=(k == KH - 1))
        meta16 = gpool.tile([P, NCH, 4], f32)
        for jj in range(NCH):
            l_sb = gwork.tile([P, 8], f32, tag="l_sb")
            nc.vector.tensor_copy(l_sb[:], ps9all[:, jj, 0:8])
            maxv = gwork.tile([P, 8], f32, tag="maxv")
            maxi = gwork.tile([P, 8], mybir.dt.uint32, tag="maxi")
            nc.vector.max_with_indices(maxv[:], maxi[:], l_sb[:])
            neg2 = gwork.tile([P, 1], f32, tag="neg2")
            nc.vector.tensor_scalar_mul(neg2[:], maxv[:, 1:2], -1.0)
            nc.vector.tensor_copy(meta16[:, jj, 0:2], maxi[:, 0:2])
            nc.scalar.activation(meta16[:, jj, 2:3], maxv[:, 0:1], AF.Sigmoid,
                                 bias=neg2[:, 0:1])
            nc.vector.tensor_scalar(meta16[:, jj, 3:4], meta16[:, jj, 2:3], -1.0, 1.0,
                                    op0=ALU.mult, op1=ALU.add)
            nc.scalar.activation(sg_all[:, jj:jj + 1], ps9all[:, jj, 8:9], AF.Sigmoid)
        nc.sync.dma_start(
            out=meta_local[:, :].rearrange("j (f c) -> f j c", c=4),
            in_=meta16[:])
        gate_ctx.close()

        # ---- partial zero-init (scalar queue; must finish before scatterbacks,
        # which start only after the first FFN block's matmuls) ----
        zinit_ctx = ExitStack()
        zp = zinit_ctx.enter_context(tc.tile_pool(name="zp", bufs=1))
        zero_sb = zp.tile([P, 2048], bf16)
        nc.vector.memset(zero_sb[:], 0.0)
        rows_per = (P * 2048) // H  # 256
        r = 0
        while r < T + 1:
            n = min(rows_per, T + 1 - r)
            if n == rows_per:
                nc.scalar.dma_start(out=partial[r:r + n, :], in_=zero_sb[:])
            else:
                nc.scalar.dma_start(out=partial[r:r + n, :], in_=zero_sb[0:1, :n * H])
            r += n
        zinit_ctx.close()

        # =================== Phase 2: AllGather of routing meta ===================
        if collectives:
            nc.gpsimd.collective_compute(
                "AllGather", ALU.bypass, replica_groups=groups,
                ins=[meta_local[:, :]], outs=[meta_all[:, :]])
        else:
            for m in range(NCORES):
                nc.gpsimd.dma_start(out=meta_all[m * NCH:(m + 1) * NCH, :],
                                    in_=meta_local[:, :])

        # =================== Phase 3: dispatch build ===================
        disp_ctx = ExitStack()
        dp = disp_ctx.enter_context(tc.tile_pool(name="disp", bufs=1))
        dw = disp_ctx.enter_context(tc.tile_pool(name="dwork", bufs=4))
        psum_d = disp_ctx.enter_context(tc.tile_pool(name="psum_d", bufs=2,
                                                     space="PSUM"))
        psum_c = disp_ctx.enter_context(tc.tile_pool(name="psum_c", bufs=1,
                                                     space="PSUM"))

        M_sb = dp.tile([P, P, 4], f32)
        nc.scalar.dma_start(out=M_sb[:], in_=meta_all[:, :])
        eb = eid_sb[:, 0:1].to_broadcast([P, P])
        m1 = dp.tile([P, P], f32)
        m2 = dp.tile([P, P], f32)
        mask = dp.tile([P, P], f32)
        w_t = dp.tile([P, P], f32)
        tmp = dp.tile([P, P], f32)
        nc.vector.tensor_tensor(out=m1[:], in0=M_sb[:, :, 0], in1=eb, op=ALU.is_equal)
        nc.vector.tensor_tensor(out=m2[:], in0=M_sb[:, :, 1], in1=eb, op=ALU.is_equal)
        nc.vector.tensor_tensor(out=mask[:], in0=m1[:], in1=m2[:], op=ALU.add)
        nc.vector.tensor_tensor(out=w_t[:], in0=m1[:], in1=M_sb[:, :, 2], op=ALU.mult)
        nc.vector.tensor_tensor(out=tmp[:], in0=m2[:], in1=M_sb[:, :, 3], op=ALU.mult)
        nc.vector.tensor_tensor(out=w_t[:], in0=w_t[:], in1=tmp[:], op=ALU.add)

        # transposes: maskT, w_tT [token, chunk]
        psA = psum_d.tile([P, P], f32, tag="psA")
        nc.tensor.transpose(out=psA[:], in_=mask[:], identity=id_f[:])
        maskT = dp.tile([P, P], f32)
        nc.vector.tensor_copy(maskT[:], psA[:])
        psB = psum_d.tile([P, P], f32, tag="psA")
        nc.tensor.matmul(out=psB[:], lhsT=lts[:], rhs=maskT[:], start=True, stop=True)
        posT = dp.tile([P, P], f32)
        nc.vector.tensor_copy(posT[:], psB[:])
        psW = psum_d.tile([P, P], f32, tag="psA")
        nc.tensor.transpose(out=psW[:], in_=w_t[:], identity=id_f[:])
        w_tT = dp.tile([P, P], f32)
        nc.vector.tensor_copy(w_tT[:], psW[:])

        # posm = posT + (1-maskT)*BIG  (invalid tokens never match a slot)
        um = dp.tile([P, P], f32)
        nc.vector.tensor_scalar(um[:], maskT[:], -float(BIGI), float(BIGI),
                                op0=ALU.mult, op1=ALU.add)
        posm = dp.tile([P, P], f32)
        nc.vector.tensor_tensor(out=posm[:], in0=posT[:], in1=um[:], op=ALU.add)

        # rhs3 [token, chunk, 3] = (id const, w, mask)
        rhs3 = dp.tile([P, P, 3], f32)
        nc.sync.dma_start(out=rhs3[:], in_=rhs3_c[:, :])
        nc.vector.tensor_copy(rhs3[:, :, 1], w_tT[:])
        nc.vector.tensor_copy(rhs3[:, :, 2], maskT[:])

        # counts, padded counts, offsets
        cnt = dp.tile([P, 1], f32)
        nc.vector.tensor_reduce(out=cnt[:], in_=mask[:], axis=X, op=ALU.add)
        cnt_oh = dp.tile([P, P], f32)
        nc.vector.tensor_tensor(out=cnt_oh[:], in0=cnt[:, 0:1].to_broadcast([P, P]),
                                in1=r128[:], op=ALU.is_equal)
        t8 = dp.tile([P, P], f32)
        nc.vector.tensor_tensor(out=t8[:], in0=cnt_oh[:], in1=tab8[:], op=ALU.mult)
        cnt8 = dp.tile([P, 1], f32)
        nc.vector.tensor_reduce(out=cnt8[:], in_=t8[:], axis=X, op=ALU.add)
        psD = psum_d.tile([P, 1], f32, tag="psD")
        nc.tensor.matmul(out=psD[:], lhsT=lts[:], rhs=cnt8[:], start=True, stop=True)
        rowoff8 = dp.tile([P, 1], f32)
        nc.vector.tensor_copy(rowoff8[:], psD[:])
        base8 = dp.tile([P, G_SC], f32)
        nc.vector.tensor_tensor(out=base8[:], in0=rowoff8[:, 0:1].to_broadcast([P, G_SC]),
                                in1=g8r[:], op=ALU.add)
        valid = dp.tile([P, G_SC], f32)
        nc.vector.tensor_tensor(out=valid[:], in0=cnt8[:, 0:1].to_broadcast([P, G_SC]),
                                in1=g8r[:], op=ALU.is_gt)
        o1 = dp.tile([P, G_SC], f32)
        nc.vector.tensor_tensor(out=o1[:], in0=base8[:], in1=valid[:], op=ALU.mult)
        o2 = dp.tile([P, G_SC], f32)
        nc.vector.tensor_scalar(o2[:], valid[:], -float(BIGI), float(BIGI),
                                op0=ALU.mult, op1=ALU.add)
        offs_f = dp.tile([P, G_SC], f32)
        nc.vector.tensor_tensor(out=offs_f[:], in0=o1[:], in1=o2[:], op=ALU.add)
        offs_i = dp.tile([P, G_SC], i32)
        nc.vector.tensor_copy(offs_i[:], offs_f[:])

        # per-chunk one-hot compaction matmuls -> psum [64, 3*P]
        ps3 = psum_c.tile([64, 3 * P], f32)
        for c in range(P):
            Sc = dw.tile([P, 64], f32, tag="Sc")
            nc.vector.tensor_tensor(out=Sc[:], in0=posm[:, c:c + 1].to_broadcast([P, 64]),
                                    in1=r64[:], op=ALU.is_equal)
            nc.tensor.matmul(out=ps3[:, 3 * c:3 * c + 3], lhsT=Sc[:],
                             rhs=rhs3[:, c, :], start=True, stop=True)
        comp3 = dp.tile([64, P, 3], f32)
        nc.vector.tensor_copy(comp3[:], ps3[:])
        # id2 = id + T*(1-hit)  (empty slots -> trash row T, weight 0)
        t1 = dp.tile([64, P], f32)
        nc.vector.tensor_scalar(t1[:], comp3[:, :, 2], -float(T), float(T),
                                op0=ALU.mult, op1=ALU.add)
        id2 = dp.tile([64, P], f32)
        nc.vector.tensor_tensor(out=id2[:], in0=comp3[:, :, 0], in1=t1[:], op=ALU.add)
        # transpose id2, w back to [chunk, slot]
        psT1 = psum_d.tile([P, 64], f32, tag="psT")
        nc.tensor.transpose(out=psT1[:], in_=id2[:], identity=id_f[0:64, 0:64])
        c2id = dp.tile([P, 64], f32)
        nc.vector.tensor_copy(c2id[:], psT1[:])
        psT2 = psum_d.tile([P, 64], f32, tag="psT")
        nc.tensor.transpose(out=psT2[:], in_=comp3[:, :, 1], identity=id_f[0:64, 0:64])
        c2w = dp.tile([P, 64], f32)
        nc.vector.tensor_copy(c2w[:], psT2[:])

        # 8+8 byte-disjoint indirect scatters (order-free; BIG offsets skipped)
        for g in range(G_SC):
            nc.gpsimd.indirect_dma_start(
                out=list_id[:, :],
                out_offset=bass.IndirectOffsetOnAxis(ap=offs_i[:, g:g + 1], axis=0),
                in_=c2id[:, 4 * g:4 * g + 4], in_offset=None,
                bounds_check=C - 4, oob_is_err=False)
        for g in range(G_SC):
            nc.gpsimd.indirect_dma_start(
                out=list_w[:, :],
                out_offset=bass.IndirectOffsetOnAxis(ap=offs_i[:, g:g + 1], axis=0),
                in_=c2w[:, 4 * g:4 * g + 4], in_offset=None,
                bounds_check=C - 4, oob_is_err=False)

        # load compacted lists back
        lm_wf = dp.tile([P, NT2], f32)
        nc.scalar.dma_start(out=lm_wf[:], in_=list_id[:, :])
        nc.vector.tensor_copy(lm_id[:], lm_wf[:])
        nc.scalar.dma_start(out=lm_w[:], in_=list_w[:, :])
        disp_ctx.close()


        # =================== shared expert (streamed weights) ===================
        def open_shared_pools(tag, sxs_bufs=2, sws_bufs=8, sw2s_bufs=4):
            sh_ctx = ExitStack()
            pools = {
                "sxs": sh_ctx.enter_context(
                    tc.tile_pool(name=f"sxs{tag}", bufs=sxs_bufs)),
                "shh": sh_ctx.enter_context(tc.tile_pool(name=f"shh{tag}", bufs=1)),
                "sws": sh_ctx.enter_context(
                    tc.tile_pool(name=f"sws{tag}", bufs=sws_bufs)),
                "sw2s": sh_ctx.enter_context(
                    tc.tile_pool(name=f"sw2s{tag}", bufs=sw2s_bufs)),
                "sev": sh_ctx.enter_context(tc.tile_pool(name=f"sev{tag}", bufs=3)),
            }
            return sh_ctx, pools

        def add_shared_psum(sh_ctx, pools, tag):
            pools["psum_s1"] = sh_ctx.enter_context(
                tc.tile_pool(name=f"psum_s1{tag}", bufs=2, space="PSUM"))
            pools["psum_sy"] = sh_ctx.enter_context(
                tc.tile_pool(name=f"psum_sy{tag}", bufs=1, space="PSUM"))

        xt_bf_r = xt_bf[:, :].rearrange("(k p) t -> p k t", p=P)    # [P, KH, TLOC]
        sw1_r = sw1[:, :].rearrange("(k p) d -> p k d", p=P)        # [P, KH, IS]
        sw3_r = sw3[:, :].rearrange("(k p) d -> p k d", p=P)
        sw2_r = sw2[:, :].rearrange("(q p) d -> p q d", p=P)        # [P, 2*KI, H]

        def shared_block(b, pools, dma_log=None):
            sxs = pools["sxs"]; shh = pools["shh"]; sws = pools["sws"]
            sw2s_p = pools["sw2s"]; sev = pools["sev"]
            psum_s1 = pools["psum_s1"]; psum_sy = pools["psum_sy"]
            bsl = slice(b * SH_TB, (b + 1) * SH_TB)
            xs = sxs.tile([P, KH, SH_TB], bf16, tag="xs")
            nc.sync.dma_start(out=xs[:], in_=xt_bf_r[:, :, bsl])
            hh = shh.tile([P, 2 * KI, SH_TB], bf16, tag="shh")
            for hs in range(2):
                i0 = hs * (IS // 2)
                for i in range(KI):
                    s1 = sws.tile([P, KH, P], bf16, tag="s1")
                    s3 = sws.tile([P, KH, P], bf16, tag="s3")
                    isl = slice(i0 + i * P, i0 + (i + 1) * P)
                    h1d = nc.sync.dma_start(out=s1[:], in_=sw1_r[:, :, isl])
                    h3d = nc.sync.dma_start(out=s3[:], in_=sw3_r[:, :, isl])
                    if dma_log is not None:
                        dma_log.append(h1d)
                        dma_log.append(h3d)
                    ps1 = psum_s1.tile([P, SH_TB], f32, tag="sps1")
                    for k in range(KH):
                        nc.tensor.matmul(out=ps1[:], lhsT=s1[:, k, :], rhs=xs[:, k, :],
                                         start=(k == 0), stop=(k == KH - 1))
                    h1 = sev.tile([P, SH_TB], bf16, tag="sh1")
                    nc.scalar.activation(h1[:], ps1[:], AF.Silu)
                    ps3b = psum_s1.tile([P, SH_TB], f32, tag="sps3")
                    for k in range(KH):
                        nc.tensor.matmul(out=ps3b[:], lhsT=s3[:, k, :], rhs=xs[:, k, :],
                                         start=(k == 0), stop=(k == KH - 1))
                    nc.vector.tensor_tensor(out=hh[:, hs * KI + i, :], in0=ps3b[:],
                                            in1=h1[:], op=ALU.mult)
            for half in range(2):
                hsl = slice(half * 512, (half + 1) * 512)
                psy = []
                for ts in range(SH_TB // P):
                    psy_t = psum_sy.tile([P, 512], f32, tag=f"psy{ts}",
                                         name=f"psy_{ts}")
                    psy.append(psy_t)
                for hs in range(2):
                    for k4 in range(KI // 4):
                        s2 = sw2s_p.tile([P, 4, 512], bf16, tag="s2")
                        nc.sync.dma_start(
                            out=s2[:],
                            in_=sw2_r[:, hs * KI + 4 * k4:hs * KI + 4 * k4 + 4, hsl])
                        for kk in range(4):
                            k = 4 * k4 + kk
                            for ts in range(SH_TB // P):
                                nc.tensor.matmul(
                                    out=psy[ts][:],
                                    lhsT=hh[:, hs * KI + k, ts * P:(ts + 1) * P],
                                    rhs=s2[:, kk, :],
                                    start=(hs == 0 and k == 0),
                                    stop=(hs == 1 and k == KI - 1))
                for ts in range(SH_TB // P):
                    ya = sev.tile([P, 512], bf16, tag="ya")
                    nc.vector.tensor_copy(ya[:], psy[ts][:])
                    r0 = b * SH_TB + ts * P
                    nc.sync.dma_start(out=yacc_d[r0:r0 + P, hsl], in_=ya[:])

        sh_ctx1, sh_pools1 = open_shared_pools("a")
        add_shared_psum(sh_ctx1, sh_pools1, "a")
        shared_block(0, sh_pools1)
        shared_block(1, sh_pools1)
        sh_ctx1.close()

        # =================== Phase 4: expert FFN ===================
        fctx = ExitStack()
        fxe = fctx.enter_context(tc.tile_pool(name="fxe", bufs=3))
        fxeT = fctx.enter_context(tc.tile_pool(name="fxeT", bufs=2))
        fh = fctx.enter_context(tc.tile_pool(name="fh", bufs=2))
        fhh = fctx.enter_context(tc.tile_pool(name="fhh", bufs=2))
        fy = fctx.enter_context(tc.tile_pool(name="fy", bufs=3))
        psum_f = fctx.enter_context(tc.tile_pool(name="psum_f", bufs=2, space="PSUM"))
        psum_t = fctx.enter_context(tc.tile_pool(name="psum_t", bufs=2, space="PSUM"))

        def ffn_hh(s0, TBb):
            j0 = s0 // P
            nts = TBb // P
            xeT = fxeT.tile([P, KH, TBb], bf16, tag="xeT")
            for ts in range(nts):
                j = j0 + ts
                xe = fxe.tile([P, H], bf16, tag="xe")
                nc.gpsimd.indirect_dma_start(
                    out=xe[:], out_offset=None,
                    in_=x_rows[:, :],
                    in_offset=bass.IndirectOffsetOnAxis(ap=lm_id[:, j:j + 1], axis=0))
                for k in range(KH):
                    pst = psum_t.tile([P, P], bf16, tag="pst")
                    nc.tensor.transpose(out=pst[:], in_=xe[:, k * P:(k + 1) * P],
                                        identity=id_bf[:])
                    nc.scalar.activation(xeT[:, k, ts * P:(ts + 1) * P], pst[:],
                                         AF.Copy)
            hh = fhh.tile([P, KI, TBb], bf16, tag="hh")
            for i in range(KI):
                isl = slice(i * P, (i + 1) * P)
                ps1 = psum_f.tile([P, TBb], f32, tag="ps1")
                for k in range(KH):
                    nc.tensor.matmul(out=ps1[:], lhsT=w1_sb[:, k, isl],
                                     rhs=xeT[:, k, :],
                                     start=(k == 0), stop=(k == KH - 1))
                h1 = fh.tile([P, TBb], bf16, tag="h1")
                nc.scalar.activation(h1[:], ps1[:], AF.Silu)
                ps3f = psum_f.tile([P, TBb], f32, tag="ps3")
                for k in range(KH):
                    nc.tensor.matmul(out=ps3f[:], lhsT=w3_sb[:, k, isl],
                                     rhs=xeT[:, k, :],
                                     start=(k == 0), stop=(k == KH - 1))
                nc.vector.tensor_tensor(out=hh[:, i, :], in0=ps3f[:], in1=h1[:],
                                        op=ALU.mult)
            return hh

        def ffn_y(s0, TBb, hh):
            j0 = s0 // P
            nts = TBb // P
            for ts in range(nts):
                j = j0 + ts
                yrow = fy.tile([P, H], bf16, tag="yrow")
                wbc = lm_w[:, j:j + 1].to_broadcast([P, 512])
                for half in range(2):
                    psy = psum_f.tile([P, 512], f32, tag="psyf")
                    for k in range(KI):
                        nc.tensor.matmul(
                            out=psy[:], lhsT=hh[:, k, ts * P:(ts + 1) * P],
                            rhs=w2_sb[:, k, half * 512:(half + 1) * 512],
                            start=(k == 0), stop=(k == KI - 1))
                    nc.vector.tensor_tensor(out=yrow[:, half * 512:(half + 1) * 512],
                                            in0=psy[:], in1=wbc, op=ALU.mult)
                nc.gpsimd.indirect_dma_start(
                    out=partial[:, :],
                    out_offset=bass.IndirectOffsetOnAxis(ap=lm_id[:, j:j + 1], axis=0),
                    in_=yrow[:], in_offset=None)

        for (s0, TBb) in FFN_BLOCKS:
            hh_b = ffn_hh(s0, TBb)
            ffn_y(s0, TBb, hh_b)
        fctx.close()
        wexp2_ctx.close()
        wexp13_ctx.close()
        sh_ctx2, sh_pools2 = open_shared_pools("b", sxs_bufs=2, sws_bufs=16,
                                               sw2s_bufs=6)

        # =================== Phase 5: ReduceScatter ===================
        rs_inst = None
        if collectives:
            rs_inst = nc.gpsimd.collective_compute(
                "ReduceScatter", ALU.add, replica_groups=groups,
                ins=[partial[0:T, :]], outs=[rs_out[:, :]])
        else:
            nc.gpsimd.dma_start(out=rs_out[:, :], in_=partial[0:TLOC, :])

        # ============ shared blocks 2,3 (hide RS) + interleaved combine ============
        cb_ctx = ExitStack()
        cb = cb_ctx.enter_context(tc.tile_pool(name="cb", bufs=4))

        def combine_chunks(jls):
            for jl in jls:
                for half in range(2):
                    hsl = slice(half * 512, (half + 1) * 512)
                    rsb = cb.tile([P, 512], bf16, tag="rsb")
                    nc.sync.dma_start(out=rsb[:], in_=rs_out[jl * P:(jl + 1) * P, hsl])
                    yab = cb.tile([P, 512], bf16, tag="yab")
                    nc.sync.dma_start(out=yab[:], in_=yacc_d[jl * P:(jl + 1) * P, hsl])
                    sgb = sg_all[:, jl:jl + 1].to_broadcast([P, 512])
                    tmul = cb.tile([P, 512], f32, tag="tmul")
                    nc.vector.tensor_tensor(out=tmul[:], in0=yab[:], in1=sgb,
                                            op=ALU.mult)
                    fin = cb.tile([P, 512], f32, tag="fin")
                    nc.vector.tensor_tensor(out=fin[:], in0=tmul[:], in1=rsb[:],
                                            op=ALU.add)
                    nc.sync.dma_start(out=out[jl * P:(jl + 1) * P, hsl], in_=fin[:])

        add_shared_psum(sh_ctx2, sh_pools2, "b")
        sh2_log = []
        shared_block(2, sh_pools2, dma_log=sh2_log)
        # Delay the RS issue until shared block 2's first 10 weight-slice pairs
        # have landed: the collective saturates the DMA subsystem for ~300 us,
        # and this lets the stream build its prefetch lookahead first.
        if rs_inst is not None and len(sh2_log) >= 20:
            from concourse.tile import add_dep_helper
            add_dep_helper(rs_inst.ins, sh2_log[19].ins, sync=True,
                           reason="RS waits for sh2 weight prefetch")
        combine_chunks(range(0, 12))
        shared_block(3, sh_pools2)
        combine_chunks(range(12, 16))
        cb_ctx.close()
        sh_ctx2.close()

    nc.finalize()
    return nc


def _host_prep(inputs):
    """Build per-core input maps from full inputs."""
    hs = _f32(inputs["hidden_states"])
    x = hs.reshape(T, H)
    gate_w = _f32(inputs["gate_w"])
    sgw = _f32(inputs["sgate_w"])
    gw9 = np.concatenate([gate_w, sgw], axis=1)       # [H, 9]
    w1 = inputs["w1"]; w3 = inputs["w3"]; w2 = inputs["w2"]

    x_rows_bf = np.zeros((T + 1, H), dtype=_bf16(np.zeros(1)).dtype)
    x_rows_bf[:T] = _bf16(x)
    xT = np.ascontiguousarray(x.T)                    # [H, T] f32
    xT_bf = _bf16(xT)
    sw1b = _bf16(inputs["sw1"]); sw3b = _bf16(inputs["sw3"])
    sw2b = _bf16(inputs["sw2"])

    in_maps = []
    for m in range(NCORES):
        sl = slice(m * TLOC, (m + 1) * TLOC)
        in_maps.append({
            "xt_f32": np.ascontiguousarray(xT[:, sl]),
            "xt_bf": np.ascontiguousarray(xT_bf[:, sl]),
            "x_rows": x_rows_bf,
            "gw9": gw9,
            "w1": _bf16(w1[m]),
            "w3": _bf16(w3[m]),
            "w2": _bf16(w2[m]),
            "sw1": sw1b,
            "sw3": sw3b,
            "sw2": sw2b,
            "eid": np.full((P, 1), float(m), dtype=np.float32),
        })
    return in_maps


def kernel(**inputs):
    global LAST_RESULT, LAST_WALL_NS
    from concourse.bass_utils import run_bass_kernel_spmd

    if "nc" not in _RUNNER:
        _RUNNER["nc"] = build_program()
    nc = _RUNNER["nc"]

    in_maps = _host_prep(inputs)
    trace = os.environ.get("KERNEL_TRACE", "0") == "1"
    import time
    t0 = time.perf_counter_ns()
    res = run_bass_kernel_spmd(nc, in_maps, list(range(NCORES)), trace=trace)
    LAST_WALL_NS = time.perf_counter_ns() - t0
    LAST_RESULT = res
    out = np.concatenate([res.results[m]["out"] for m in range(NCORES)], axis=0)
    return out.reshape(B, S, H).astype(np.float32)


if __name__ == "__main__":
    nc = build_program()
    print("program built ok")
